# revision 18
# baseline (speedup 1.0000x reference)
"""2-layer GCN (GCNConv x2, symmetric norm, self-loops) on 8 Trainium2 NeuronCores.

Strategy (graph/data parallel, v2):
  - Nodes are partitioned contiguously across 8 cores (6250/core, padded to
    6272 = 49*128 slots; sequential pos, no permutation). Blocks 0..24 are the
    "lo" half (3200 slots), 25..48 the "hi" half (3072) -- each half's global
    gather table fits int16 row indices (8*3200 = 25600 < 32768).
  - Layer 1 transform h = x @ W1 is row-sharded; xt is host-packed so each
    4-block group loads with one 4 MB DMA (32 KB/partition descriptors).
  - h is AllGather'd in TWO collectives (lo half fired mid-phase-A, hi at the
    end) so the AG overlaps the tail of the transform and the head of the
    aggregation (lo gathers only need the lo AG).
  - Aggregation: per 2-block gather-superblock, dma_gather fetches h[src] rows
    (512 B each) chunk-wise; a one-hot matrix M (built on-device from per-slot
    dst_local/norm via one bf16 tensor_scalar) scatter-adds them on the
    TensorEngine with PSUM accumulation. Chunk counts are per-block (max over
    the 8 cores, since all cores share one program), cutting padded rows ~12%
    vs a global max. Self-loop edges form one diagonal chunk per block whose
    "gather" is a plain contiguous dma_start from h_own (no SWDGE descriptors).
  - bias+ReLU (ScalarE), transform by W2, then g is AllGather'd (again split
    lo/hi) and the same chunk structure aggregates layer 2.

kernel(**inputs) takes full unsharded inputs, returns the full [50000, 128]
output. Self-contained: no sibling imports; /opt/trn_rl_repo provides bass.
"""

import math
import os
import sys

import numpy as np

sys.path.insert(0, "/opt/trn_rl_repo")

import concourse.bass as bass  # noqa: E402
import concourse.mybir as mybir  # noqa: E402
import concourse.tile as tile  # noqa: E402
from concourse import bacc  # noqa: E402

P = 128
NCORES = 8
GA = 4    # blocks per phase-A matmul group (one xt DMA each)
GSB = 2   # blocks per gather superblock
GMAX = 6  # chunks (of 128 idxs) per dma_gather op (768-idx HW cap)

F32 = mybir.dt.float32
BF16 = mybir.dt.bfloat16
I16 = mybir.dt.int16


def _swizzle_idx(idx):
    """gather idx j -> [j%16, j//16], replicated across the 8 groups of 16."""
    n = idx.shape[0]
    a = np.zeros((16, n // 16), np.int16)
    a[np.arange(n) % 16, np.arange(n) // 16] = idx.astype(np.int16)
    return np.tile(a, (8, 1))


# ---------------------------------------------------------------------------
# host-side preprocessing
# ---------------------------------------------------------------------------

def _prep(x, edge_index, W1, b1, W2, b2):
    import ml_dtypes

    N, F_in = x.shape
    F_h = W1.shape[1]
    F_out = W2.shape[1]
    assert N % NCORES == 0 and F_in % P == 0 and F_h == 2 * P and F_out == P
    npc_raw = N // NCORES
    nb = math.ceil(npc_raw / P)
    nb_lo = (nb + 1) // 2
    nb_hi = nb - nb_lo
    npc = nb * P
    npc_lo, npc_hi = nb_lo * P, nb_hi * P
    ntot_lo, ntot_hi = NCORES * npc_lo, NCORES * npc_hi
    assert ntot_lo < 2 ** 15 and ntot_hi < 2 ** 15
    kt = F_in // P

    src = np.asarray(edge_index[0]).astype(np.int64)
    dst = np.asarray(edge_index[1]).astype(np.int64)
    deg = np.bincount(dst, minlength=N).astype(np.float64) + 1.0  # + self loop
    dinv = 1.0 / np.sqrt(deg)
    nrm_e = (dinv[src] * dinv[dst]).astype(np.float32)

    core_s, loc_s = src // npc_raw, src % npc_raw
    core_d, loc_d = dst // npc_raw, dst % npc_raw
    hi_s = (loc_s // P) >= nb_lo
    row_s = np.where(hi_s, core_s * npc_hi + (loc_s - npc_lo),
                     core_s * npc_lo + loc_s)
    blk_d = loc_d // P
    dl_d = (loc_d % P).astype(np.float32)

    # per-(core, block, half) counts -> per-block chunk counts (max over cores)
    cnt = np.zeros((NCORES, nb, 2), np.int64)
    np.add.at(cnt, (core_d, blk_d, hi_s.astype(np.int64)), 1)
    cl = np.ceil(cnt[:, :, 0].max(axis=0) / P).astype(int)  # [nb]
    ch = np.ceil(cnt[:, :, 1].max(axis=0) / P).astype(int)

    gsbs = [tuple(range(s, min(s + GSB, nb))) for s in range(0, nb, GSB)]

    # msg-tile layout per gsb: [lo chunks of blocks...][hi chunks...][selfs]
    # global chunk index (for mdst/mnorm) == cbase + msg column.
    info = []
    cbase = 0
    lo_ch_cum = 0   # lo chunks emitted so far (ilo column base / 8)
    hi_ch_cum = 0
    for blocks in gsbs:
        Lg = int(sum(cl[b] for b in blocks))
        Hg = int(sum(ch[b] for b in blocks))
        lo_cols, hi_cols, self_col = {}, {}, {}
        off = 0
        for b in blocks:
            hi_cols[b] = (off, int(ch[b]))
            off += int(ch[b])
        for b in blocks:
            lo_cols[b] = (off, int(cl[b]))
            off += int(cl[b])
        for b in blocks:
            self_col[b] = off
            off += 1
        info.append(dict(blocks=blocks, Lg=Lg, Hg=Hg, Cg=off, cbase=cbase,
                         lo_cols=lo_cols, hi_cols=hi_cols, self_col=self_col,
                         lo_icol=lo_ch_cum * 8, hi_icol=hi_ch_cum * 8))
        cbase += off
        lo_ch_cum += Lg
        hi_ch_cum += Hg
    # pad to a multiple of 8 f32 columns: per-partition row stride must stay
    # 16/32-byte aligned or the staged-input DMA faults on device
    nchunks = (cbase + 7) // 8 * 8
    n_lo_ch, n_hi_ch = lo_ch_cum, hi_ch_cum

    # per-block slot base inside the per-half idx streams
    lo_slot_base = np.concatenate([[0], np.cumsum(cl)]) * P
    hi_slot_base = np.concatenate([[0], np.cumsum(ch)]) * P
    # msg column of (b, half) chunks, global chunk order for mdst/mnorm
    colof = {}
    for g in info:
        for b in g['blocks']:
            colof[(b, 0)] = g['cbase'] + g['lo_cols'][b][0]
            colof[(b, 1)] = g['cbase'] + g['hi_cols'][b][0]
            colof[(b, 2)] = g['cbase'] + g['self_col'][b]

    order_half = np.argsort(hi_s * N * 2 + core_d * npc + blk_d, kind='stable')

    cores = []
    for c in range(NCORES):
        mask = core_d == c
        e_row = row_s[mask]
        e_hi = hi_s[mask]
        e_blk = blk_d[mask]
        e_dl = dl_d[mask]
        e_nrm = nrm_e[mask]

        idx_lo = np.zeros(n_lo_ch * P, np.int64)
        idx_hi = np.zeros(n_hi_ch * P, np.int64)
        mdst = np.zeros((P, nchunks), np.float32)
        mnorm = np.zeros((P, nchunks), np.float32)

        for h, idx_arr, sbase, c_arr in ((0, idx_lo, lo_slot_base, cl),
                                         (1, idx_hi, hi_slot_base, ch)):
            sel = e_hi == (h == 1)
            rr, bb = e_row[sel], e_blk[sel]
            dd, nn = e_dl[sel], e_nrm[sel]
            o = np.argsort(bb, kind='stable')
            rr, bb, dd, nn = rr[o], bb[o], dd[o], nn[o]
            start = np.searchsorted(bb, np.arange(nb))
            end = np.searchsorted(bb, np.arange(nb) + 1)
            for b in range(nb):
                k = end[b] - start[b]
                assert k <= c_arr[b] * P
                sl = slice(start[b], end[b])
                idx_arr[sbase[b]: sbase[b] + k] = rr[sl]
                col0 = colof[(b, h)]
                fd = np.zeros(c_arr[b] * P, np.float32)
                fn = np.zeros(c_arr[b] * P, np.float32)
                fd[:k] = dd[sl]
                fn[:k] = nn[sl]
                mdst[:, col0:col0 + c_arr[b]] = fd.reshape(c_arr[b], P).T
                mnorm[:, col0:col0 + c_arr[b]] = fn.reshape(c_arr[b], P).T

        # self-loop diagonal chunks
        for b in range(nb):
            col = colof[(b, 2)]
            mdst[:, col] = np.arange(P, dtype=np.float32)
            n_real = min(P, npc_raw - b * P)
            nodes = c * npc_raw + b * P + np.arange(n_real)
            d2 = (dinv[nodes] ** 2).astype(np.float32)
            mnorm[:n_real, col] = d2

        assert idx_lo.min() >= 0 and idx_lo.max() < ntot_lo
        assert idx_hi.min() >= 0 and (n_hi_ch == 0 or idx_hi.max() < ntot_hi)

        # x slice, padded/transposed/group-packed: per phase-A group of GA
        # blocks, [P, kt*wg] contiguous (32 KB/partition descriptors)
        nodes = np.arange(c * npc_raw, (c + 1) * npc_raw)
        xp = np.zeros((npc, F_in), np.float32)
        xp[:npc_raw] = np.asarray(x[nodes], np.float32)
        xt3 = np.ascontiguousarray(xp.T).reshape(kt, P, npc)
        parts = []
        for g0 in range(0, nb, GA):
            c0, c1 = g0 * P, min(g0 + GA, nb) * P
            parts.append(xt3[:, :, c0:c1].transpose(1, 0, 2).reshape(P, -1))
        xt = np.concatenate(parts, axis=1).astype(ml_dtypes.bfloat16)

        cores.append({
            "xt": xt,
            "idx_lo": _swizzle_idx(idx_lo),
            "idx_hi": _swizzle_idx(idx_hi),
            "mdst": mdst,
            "mnorm": mnorm,
        })

    iota = np.tile(np.arange(P, dtype=np.float32)[None, :], (P, 1))
    if os.environ.get("GNN_F32_IOTA", "0") != "1":
        iota = iota.astype(ml_dtypes.bfloat16)
    shared = {
        "w1": np.asarray(W1, np.float32).astype(ml_dtypes.bfloat16),
        "w2": np.asarray(W2, np.float32),
        "b1p": np.asarray(b1, np.float32).reshape(2, P).T.copy(),
        "b2b": np.tile(np.asarray(b2, np.float32)[None, :], (P, 1)),
        "iota": iota,
    }
    cfg = dict(N=N, F_in=F_in, F_h=F_h, F_out=F_out, npc_raw=npc_raw, nb=nb,
               nb_lo=nb_lo, nb_hi=nb_hi, npc=npc, npc_lo=npc_lo,
               npc_hi=npc_hi, ntot_lo=ntot_lo, ntot_hi=ntot_hi, kt=kt,
               nchunks=nchunks, n_lo_ch=n_lo_ch, n_hi_ch=n_hi_ch, info=info,
               cl=cl.tolist(), ch=ch.tolist())
    return cfg, cores, shared


# ---------------------------------------------------------------------------
# device kernel
# ---------------------------------------------------------------------------

def _build_nc(cfg):
    F_in, F_h, F_out = cfg["F_in"], cfg["F_h"], cfg["F_out"]
    nb, nb_lo, npc = cfg["nb"], cfg["nb_lo"], cfg["npc"]
    npc_lo, npc_hi = cfg["npc_lo"], cfg["npc_hi"]
    ntot_lo, ntot_hi = cfg["ntot_lo"], cfg["ntot_hi"]
    kt, nchunks = cfg["kt"], cfg["nchunks"]
    n_lo_ch, n_hi_ch = cfg["n_lo_ch"], cfg["n_hi_ch"]
    info = cfg["info"]
    rg = [list(range(NCORES))]

    nc = bacc.Bacc(None, num_devices=NCORES, num_swdge_queues=4)

    xt_d = nc.declare_dram_parameter("xt", [P, kt * npc], BF16, isOutput=False)
    w1_d = nc.declare_dram_parameter("w1", [F_in, F_h], BF16, isOutput=False)
    w2_d = nc.declare_dram_parameter("w2", [F_h, F_out], F32, isOutput=False)
    b1_d = nc.declare_dram_parameter("b1p", [P, 2], F32, isOutput=False)
    b2_d = nc.declare_dram_parameter("b2b", [P, F_out], F32, isOutput=False)
    no_tables = os.environ.get("GNN_NO_TABLES", "0") == "1"
    tab_sel = set(os.environ.get("GNN_TABLES",
                                 "iota,ilo,ihi,mdst,mnorm").split(","))
    IOTA_DT = F32 if os.environ.get("GNN_F32_IOTA", "0") == "1" else BF16
    if not no_tables:
        if "iota" in tab_sel:
            iota_d = nc.declare_dram_parameter("iota", [P, P], IOTA_DT,
                                               isOutput=False)
        if "ilo" in tab_sel:
            ilo_d = nc.declare_dram_parameter("idx_lo", [P, n_lo_ch * 8], I16,
                                              isOutput=False)
        if "ihi" in tab_sel:
            ihi_d = nc.declare_dram_parameter("idx_hi", [P, max(n_hi_ch, 1) * 8],
                                              I16, isOutput=False)
        if "mdst" in tab_sel:
            mdst_d = nc.declare_dram_parameter("mdst", [P, nchunks], F32,
                                               isOutput=False)
        if "mnorm" in tab_sel:
            mnorm_d = nc.declare_dram_parameter("mnorm", [P, nchunks], F32,
                                                isOutput=False)
    out_d = nc.declare_dram_parameter("out", [npc, F_out], F32, isOutput=True)

    with tile.TileContext(nc) as tc:
        with (
            tc.tile_pool(name="const", bufs=1) as const,
            tc.tile_pool(name="dram", bufs=1, space="DRAM") as dram,
        ):
            h_own_l = dram.tile([npc_lo, F_h], BF16)
            h_own_h = dram.tile([npc_hi, F_h], BF16)
            h_full_l = dram.tile([ntot_lo, F_h], BF16, addr_space="Shared")
            h_full_h = dram.tile([ntot_hi, F_h], BF16, addr_space="Shared")
            g_own_l = dram.tile([npc_lo, F_out], BF16)
            g_own_h = dram.tile([npc_hi, F_out], BF16)
            g_full_l = dram.tile([ntot_lo, F_out], BF16, addr_space="Shared")
            g_full_h = dram.tile([ntot_hi, F_out], BF16, addr_space="Shared")

            w1_t = const.tile([P, kt, F_h], BF16)
            w2_t = const.tile([P, 2, F_out], F32)
            b1_t = const.tile([P, 2], F32)
            b2_t = const.tile([P, F_out], F32)
            if not no_tables:
                if "iota" in tab_sel:
                    iota_t = const.tile([P, P], IOTA_DT)
                if "ilo" in tab_sel:
                    ilo_t = const.tile([P, n_lo_ch * 8], I16)
                if "ihi" in tab_sel:
                    ihi_t = const.tile([P, max(n_hi_ch, 1) * 8], I16)
                if "mdst" in tab_sel:
                    mdst_t = const.tile([P, nchunks], F32)
                if "mnorm" in tab_sel:
                    mnorm_t = const.tile([P, nchunks], F32)

            nc.sync.dma_start(w1_t[:], w1_d[:].rearrange("(a p) o -> p a o", p=P))
            nc.sync.dma_start(w2_t[:], w2_d[:].rearrange("(h p) o -> p h o", p=P))
            nc.sync.dma_start(b1_t[:], b1_d[:])
            nc.sync.dma_start(b2_t[:], b2_d[:])
            if not no_tables:
                if "iota" in tab_sel:
                    nc.sync.dma_start(iota_t[:], iota_d[:])
                if "ilo" in tab_sel:
                    nc.sync.dma_start(ilo_t[:], ilo_d[:])
                if "ihi" in tab_sel:
                    nc.sync.dma_start(ihi_t[:], ihi_d[:])
                if "mdst" in tab_sel:
                    nc.sync.dma_start(mdst_t[:], mdst_d[:])
                if "mnorm" in tab_sel:
                    nc.sync.dma_start(mnorm_t[:], mnorm_d[:])


            def maybe_ag(in_t, out_t):
                if os.environ.get("GNN_NO_CC", "0") == "1":
                    return
                nc.gpsimd.collective_compute(
                    "AllGather", mybir.AluOpType.bypass, replica_groups=rg,
                    ins=[in_t[:]], outs=[out_t[:]])
            def h_dest(b):
                if b < nb_lo:
                    return h_own_l[b * P:(b + 1) * P, :]
                return h_own_h[(b - nb_lo) * P:(b - nb_lo + 1) * P, :]

            def g_dest(b):
                if b < nb_lo:
                    return g_own_l[b * P:(b + 1) * P, :]
                return g_own_h[(b - nb_lo) * P:(b - nb_lo + 1) * P, :]

            # ---- phase A: h = x @ W1 (one 4 MB contiguous DMA per group) ----
            psumA = tc.tile_pool(name="psumA", bufs=1, space="PSUM")
            psum = psumA.__enter__()
            workA = tc.tile_pool(name="workA", bufs=1)
            wa = workA.__enter__()
            ag_a_done = False
            offs, o = {}, 0
            for g0 in range(0, nb, GA):
                offs[g0] = o
                o += kt * len(range(g0, min(g0 + GA, nb))) * P
            for g0 in list(range(0, nb, GA))[::-1]:
                gb = list(range(g0, min(g0 + GA, nb)))
                wg = len(gb) * P
                off = offs[g0]
                xt_t = wa.tile([P, kt * wg], BF16,
                               tag=f"xt{len(gb)}", bufs=3 if len(gb) == GA else 1)
                nc.sync.dma_start(xt_t[:], xt_d[:, off:off + kt * wg])
                phs = [psum.tile([P, F_h], F32, tag="ph", bufs=2 * GA,
                                 space="PSUM", name=f"ph{g0}_{i}")
                       for i in range(len(gb))]
                for a in range(kt):
                    for i in range(len(gb)):
                        nc.tensor.matmul(phs[i][:],
                                         lhsT=xt_t[:, a * wg + i * P:
                                                   a * wg + (i + 1) * P],
                                         rhs=w1_t[:, a, :],
                                         start=(a == 0), stop=(a == kt - 1))
                for i, b in enumerate(gb):
                    h_sb = wa.tile([P, F_h], BF16, tag="hsb", bufs=3)
                    nc.vector.tensor_copy(h_sb[:], phs[i][:])
                    nc.sync.dma_start(h_dest(b), h_sb[:])
                if (not ag_a_done and gb[0] <= nb_lo
                        and os.environ.get("GNN_NO_MID_AG", "0") != "1"):
                    maybe_ag(h_own_h, h_full_h)
                    ag_a_done = True
            workA.__exit__(None, None, None)
            psumA.__exit__(None, None, None)
            if not ag_a_done:
                maybe_ag(h_own_h, h_full_h)
            maybe_ag(h_own_l, h_full_l)

            psumC = tc.tile_pool(name="psumC", bufs=1, space="PSUM")
            psum = psumC.__enter__()
            workC = tc.tile_pool(name="workC", bufs=1)
            wc = workC.__enter__()
            qn = [0]

            def gathers(dst_t, c0, nch, table, idx_t, icol0, elem):
                for s in range(0, nch, GMAX):
                    k = min(GMAX, nch - s)
                    nc.gpsimd.dma_gather(
                        out_ap=dst_t[:, c0 + s:c0 + s + k, :], in_ap=table,
                        idxs_ap=idx_t[:, icol0 + s * 8:icol0 + (s + k) * 8],
                        num_idxs=k * P, num_idxs_reg=k * P, elem_size=elem,
                        queue_num=qn[0] % 4)
                    qn[0] += 1

            def m_tile(gc):
                m = wc.tile([P, P], BF16, tag="m", bufs=8)
                nc.vector.tensor_scalar(
                    out=m[:], in0=iota_t[:],
                    scalar1=mdst_t[:, gc:gc + 1], scalar2=mnorm_t[:, gc:gc + 1],
                    op0=mybir.AluOpType.is_equal, op1=mybir.AluOpType.mult)
                return m

            # ---- phase C: aggregate layer 1, relu, transform by W2 ----
            skip_c = os.environ.get("GNN_SKIP_C", "0") == "1"
            skip_e = os.environ.get("GNN_SKIP_E", "0") == "1"
            ag2_a_done = False
            for g in (() if skip_c else info[::-1]):
                blocks, Cg, cb = g["blocks"], g["Cg"], g["cbase"]
                msg = wc.tile([P, Cg, F_h], BF16, tag="msg", bufs=5,
                              name=f"msg{blocks[0]}")
                gathers(msg, 0, g["Hg"], h_full_h[:], ihi_t, g["hi_icol"], F_h)
                gathers(msg, g["Hg"], g["Lg"], h_full_l[:], ilo_t,
                        g["lo_icol"], F_h)
                for b in blocks:
                    nc.sync.dma_start(msg[:, g["self_col"][b], :], h_dest(b))
                for b in blocks:
                    pa = psum.tile([P, P], F32, tag="pa", bufs=2, space="PSUM")
                    pb = psum.tile([P, P], F32, tag="pb", bufs=2, space="PSUM")
                    l0, lk = g["lo_cols"][b]
                    h0, hk = g["hi_cols"][b]
                    chunks = (list(range(h0, h0 + hk)) +
                              list(range(l0, l0 + lk)) + [g["self_col"][b]])
                    for ci, ccol in enumerate(chunks):
                        m = m_tile(cb + ccol)
                        st, sp = ci == 0, ci == len(chunks) - 1
                        nc.tensor.matmul(pa[:], lhsT=msg[:, ccol, 0:P],
                                         rhs=m[:], start=st, stop=sp)
                        nc.tensor.matmul(pb[:], lhsT=msg[:, ccol, P:F_h],
                                         rhs=m[:], start=st, stop=sp)
                    ra = wc.tile([P, P], F32, tag="ra", bufs=2)
                    rb = wc.tile([P, P], F32, tag="rb", bufs=2)
                    nc.scalar.activation(ra[:], pa[:],
                                         mybir.ActivationFunctionType.Relu,
                                         bias=b1_t[:, 0:1], scale=1.0)
                    nc.scalar.activation(rb[:], pb[:],
                                         mybir.ActivationFunctionType.Relu,
                                         bias=b1_t[:, 1:2], scale=1.0)
                    pg = psum.tile([P, F_out], F32, tag="pgo", bufs=2,
                                   space="PSUM")
                    nc.tensor.matmul(pg[:], lhsT=ra[:], rhs=w2_t[:, 0, :],
                                     start=True, stop=False)
                    nc.tensor.matmul(pg[:], lhsT=rb[:], rhs=w2_t[:, 1, :],
                                     start=False, stop=True)
                    g_sb = wc.tile([P, F_out], BF16, tag="gsb", bufs=3)
                    nc.vector.tensor_copy(g_sb[:], pg[:])
                    nc.sync.dma_start(g_dest(b), g_sb[:])
                if (not ag2_a_done and blocks[0] <= nb_lo
                        and os.environ.get("GNN_NO_MID_AG", "0") != "1"):
                    maybe_ag(g_own_h, g_full_h)
                    ag2_a_done = True
            if not ag2_a_done:
                maybe_ag(g_own_h, g_full_h)
            maybe_ag(g_own_l, g_full_l)

            # ---- phase E: aggregate layer 2, add bias, write out ----
            for g in (() if skip_e else info):
                blocks, Cg, cb = g["blocks"], g["Cg"], g["cbase"]
                msg2 = wc.tile([P, Cg, F_out], BF16, tag="msg2", bufs=5,
                               name=f"msg2_{blocks[0]}")
                gathers(msg2, 0, g["Hg"], g_full_h[:], ihi_t, g["hi_icol"],
                        F_out)
                gathers(msg2, g["Hg"], g["Lg"], g_full_l[:], ilo_t,
                        g["lo_icol"], F_out)
                for b in blocks:
                    nc.sync.dma_start(msg2[:, g["self_col"][b], :], g_dest(b))
                for b in blocks:
                    po = psum.tile([P, F_out], F32, tag="pgo", bufs=2,
                                   space="PSUM")
                    l0, lk = g["lo_cols"][b]
                    h0, hk = g["hi_cols"][b]
                    chunks = (list(range(h0, h0 + hk)) +
                              list(range(l0, l0 + lk)) + [g["self_col"][b]])
                    for ci, ccol in enumerate(chunks):
                        m = m_tile(cb + ccol)
                        nc.tensor.matmul(po[:], lhsT=m[:], rhs=msg2[:, ccol, :],
                                         start=(ci == 0),
                                         stop=(ci == len(chunks) - 1))
                    o_sb = wc.tile([P, F_out], F32, tag="osb", bufs=3)
                    nc.vector.tensor_tensor(out=o_sb[:], in0=po[:], in1=b2_t[:],
                                            op=mybir.AluOpType.add)
                    nc.sync.dma_start(out_d[b * P:(b + 1) * P, :], o_sb[:])
            if skip_e:
                for b in range(nb):
                    o_sb = wc.tile([P, F_out], F32, tag="osb", bufs=3)
                    nc.vector.tensor_copy(o_sb[:], b2_t[:])
                    nc.sync.dma_start(out_d[b * P:(b + 1) * P, :], o_sb[:])
            workC.__exit__(None, None, None)
            psumC.__exit__(None, None, None)

    nc.compile()
    return nc


def _in_maps(cfg, cores, shared):
    maps = [{**shared, **c} for c in cores]
    sel = set(os.environ.get("GNN_TABLES", "iota,ilo,ihi,mdst,mnorm").split(","))
    keymap = {"iota": "iota", "ilo": "idx_lo", "ihi": "idx_hi",
              "mdst": "mdst", "mnorm": "mnorm"}
    for m in maps:
        for short, key in keymap.items():
            if os.environ.get("GNN_NO_TABLES", "0") == "1" or short not in sel:
                m.pop(key, None)
    return maps


def _assemble(cfg, outs):
    N, F_out, npc_raw = cfg["N"], cfg["F_out"], cfg["npc_raw"]
    full = np.empty((N, F_out), np.float32)
    for c in range(NCORES):
        full[c * npc_raw:(c + 1) * npc_raw] = outs[c][:npc_raw]
    return full


# ---------------------------------------------------------------------------
# entry points
# ---------------------------------------------------------------------------

def kernel(x, edge_index, W1, b1, W2, b2):
    cfg, cores, shared = _prep(x, edge_index, W1, b1, W2, b2)
    nc = _build_nc(cfg)
    from concourse.bass_utils import run_bass_kernel_spmd
    res = run_bass_kernel_spmd(nc, _in_maps(cfg, cores, shared),
                               list(range(NCORES)))
    return _assemble(cfg, [r["out"] for r in res.results])


def run_profiled(x, edge_index, W1, b1, W2, b2, tmpdir=None):
    """Like kernel(), but traces on HW; returns (out, exec_time_ns, tmpdir)."""
    import time

    t0 = time.time()
    cfg, cores, shared = _prep(x, edge_index, W1, b1, W2, b2)
    print(f"prep {time.time() - t0:.1f}s; chunks={cfg['nchunks']} "
          f"lo_ch={cfg['n_lo_ch']} hi_ch={cfg['n_hi_ch']}")
    t0 = time.time()
    nc = _build_nc(cfg)
    print(f"build {time.time() - t0:.1f}s; {len(nc.inst_map)} instructions")
    from concourse.bass_utils import run_bass_kernel_spmd
    in_maps = _in_maps(cfg, cores, shared)
    t0 = time.time()
    res = run_bass_kernel_spmd(nc, in_maps, list(range(NCORES)))
    print(f"run {time.time() - t0:.1f}s")
    out = _assemble(cfg, [r["out"] for r in res.results])
    exec_ns = None
    try:
        t0 = time.time()
        res2 = run_bass_kernel_spmd(nc, in_maps, list(range(NCORES)),
                                    trace=True, tmpdir=tmpdir)
        print(f"traced run {time.time() - t0:.1f}s")
        exec_ns = res2.exec_time_ns
    except Exception as e:
        print(f"trace run failed: {type(e).__name__}: {str(e)[:200]}")
    return out, exec_ns, tmpdir


def _numpy_ref(x, edge_index, W1, b1, W2, b2):
    N = x.shape[0]
    src = np.concatenate([edge_index[0], np.arange(N)])
    dst = np.concatenate([edge_index[1], np.arange(N)])
    deg = np.bincount(dst, minlength=N).astype(np.float64)
    dinv = np.where(deg > 0, 1 / np.sqrt(deg), 0)
    nrm = (dinv[src] * dinv[dst]).astype(np.float32)

    def layer(h, W, b):
        hw = h @ W
        out = np.zeros((N, W.shape[1]), np.float32)
        np.add.at(out, dst, hw[src] * nrm[:, None])
        return out + b

    h = np.maximum(layer(x, W1, b1), 0)
    return layer(h, W2, b2)


def _selftest_sim():
    from concourse import bass_interp
    rng = np.random.default_rng(1)
    N, E, F_in = 2048, 8192, 512
    x = rng.standard_normal((N, F_in), dtype=np.float32)
    ei = rng.integers(0, N, (2, E)).astype(np.int64)
    W1 = (rng.standard_normal((F_in, 256), dtype=np.float32) * F_in ** -0.5)
    W2 = (rng.standard_normal((256, 128), dtype=np.float32) * 256 ** -0.5)
    b1 = rng.standard_normal(256).astype(np.float32) * 0.1
    b2 = rng.standard_normal(128).astype(np.float32) * 0.1

    cfg, cores, shared = _prep(x, ei, W1, b1, W2, b2)
    print("cfg:", {k: v for k, v in cfg.items() if k not in ("info",)})
    nc = _build_nc(cfg)
    print("built; instructions:", len(nc.inst_map))

    sim = bass_interp.MultiCoreSim(nc, NCORES)
    for i, m in enumerate(_in_maps(cfg, cores, shared)):
        for k, v in m.items():
            sim.cores[i].tensor(k)[:] = v
    sim.simulate()
    outs = [np.array(sim.cores[i].mem_tensor("out")) for i in range(NCORES)]
    got = _assemble(cfg, outs)
    want = _numpy_ref(x, ei, W1, b1, W2, b2)
    err = np.abs(got - want).max() / (np.abs(want).max() + 1e-9)
    print("selftest rel err:", err)
    assert err < 1e-2, "selftest FAILED"
    print("SELFTEST PASSED")


if __name__ == "__main__":
    _selftest_sim()


# revision 25
# speedup vs baseline: 1.2695x; 1.2695x over previous
"""2-layer GCN (GCNConv x2, symmetric norm, self-loops) on 8 Trainium2 NeuronCores.

Strategy (graph/data parallel, v2):
  - Nodes are partitioned contiguously across 8 cores (6250/core, padded to
    6272 = 49*128 slots; sequential pos, no permutation). Blocks 0..24 are the
    "lo" half (3200 slots), 25..48 the "hi" half (3072) -- each half's global
    gather table fits int16 row indices (8*3200 = 25600 < 32768).
  - Layer 1 transform h = x @ W1 is row-sharded; xt is host-packed so each
    4-block group loads with one 4 MB DMA (32 KB/partition descriptors).
  - h is AllGather'd in TWO collectives (lo half fired mid-phase-A, hi at the
    end) so the AG overlaps the tail of the transform and the head of the
    aggregation (lo gathers only need the lo AG).
  - Aggregation: per 2-block gather-superblock, dma_gather fetches h[src] rows
    (512 B each) chunk-wise; a one-hot matrix M (built on-device from per-slot
    dst_local/norm via one bf16 tensor_scalar) scatter-adds them on the
    TensorEngine with PSUM accumulation. Chunk counts are per-block (max over
    the 8 cores, since all cores share one program), cutting padded rows ~12%
    vs a global max. Self-loop edges form one diagonal chunk per block whose
    "gather" is a plain contiguous dma_start from h_own (no SWDGE descriptors).
  - bias+ReLU (ScalarE), transform by W2, then g is AllGather'd (again split
    lo/hi) and the same chunk structure aggregates layer 2.

kernel(**inputs) takes full unsharded inputs, returns the full [50000, 128]
output. Self-contained: no sibling imports; /opt/trn_rl_repo provides bass.
"""

import math
import os
import sys

import numpy as np

sys.path.insert(0, "/opt/trn_rl_repo")

import concourse.bass as bass  # noqa: E402
import concourse.mybir as mybir  # noqa: E402
import concourse.tile as tile  # noqa: E402
from concourse import bacc  # noqa: E402

P = 128
NCORES = 8
GA = 4    # blocks per phase-A matmul group (one xt DMA each)
GSB = 2   # blocks per gather superblock
GMAX = 6  # chunks (of 128 idxs) per dma_gather op (768-idx HW cap)

F32 = mybir.dt.float32
BF16 = mybir.dt.bfloat16
I16 = mybir.dt.int16


def _swizzle_idx(idx):
    """gather idx j -> [j%16, j//16], replicated across the 8 groups of 16."""
    n = idx.shape[0]
    a = np.zeros((16, n // 16), np.int16)
    a[np.arange(n) % 16, np.arange(n) // 16] = idx.astype(np.int16)
    return np.tile(a, (8, 1))


# ---------------------------------------------------------------------------
# host-side preprocessing
# ---------------------------------------------------------------------------

def _prep(x, edge_index, W1, b1, W2, b2):
    import ml_dtypes

    N, F_in = x.shape
    F_h = W1.shape[1]
    F_out = W2.shape[1]
    assert N % NCORES == 0 and F_in % P == 0 and F_h == 2 * P and F_out == P
    npc_raw = N // NCORES
    nb = math.ceil(npc_raw / P)
    nb_lo = (nb + 1) // 2
    nb_hi = nb - nb_lo
    npc = nb * P
    npc_lo, npc_hi = nb_lo * P, nb_hi * P
    ntot_lo, ntot_hi = NCORES * npc_lo, NCORES * npc_hi
    assert ntot_lo < 2 ** 15 and ntot_hi < 2 ** 15
    kt = F_in // P

    src = np.asarray(edge_index[0]).astype(np.int64)
    dst = np.asarray(edge_index[1]).astype(np.int64)
    deg = np.bincount(dst, minlength=N).astype(np.float64) + 1.0  # + self loop
    dinv = 1.0 / np.sqrt(deg)
    nrm_e = (dinv[src] * dinv[dst]).astype(np.float32)

    core_s, loc_s = src // npc_raw, src % npc_raw
    core_d, loc_d = dst // npc_raw, dst % npc_raw
    hi_s = (loc_s // P) >= nb_lo
    row_s = np.where(hi_s, core_s * npc_hi + (loc_s - npc_lo),
                     core_s * npc_lo + loc_s)
    blk_d = loc_d // P
    dl_d = (loc_d % P).astype(np.float32)

    # per-(core, block, half) counts -> per-block chunk counts (max over cores)
    cnt = np.zeros((NCORES, nb, 2), np.int64)
    np.add.at(cnt, (core_d, blk_d, hi_s.astype(np.int64)), 1)
    cl = np.ceil(cnt[:, :, 0].max(axis=0) / P).astype(int)  # [nb]
    ch = np.ceil(cnt[:, :, 1].max(axis=0) / P).astype(int)

    gsbs = [tuple(range(s, min(s + GSB, nb))) for s in range(0, nb, GSB)]

    # msg-tile layout per gsb: [lo chunks of blocks...][hi chunks...][selfs]
    # global chunk index (for mdst/mnorm) == cbase + msg column.
    info = []
    cbase = 0
    lo_ch_cum = 0   # lo chunks emitted so far (ilo column base / 8)
    hi_ch_cum = 0
    for blocks in gsbs:
        Lg = int(sum(cl[b] for b in blocks))
        Hg = int(sum(ch[b] for b in blocks))
        lo_cols, hi_cols, self_col = {}, {}, {}
        off = 0
        for b in blocks:
            lo_cols[b] = (off, int(cl[b]))
            off += int(cl[b])
        for b in blocks:
            hi_cols[b] = (off, int(ch[b]))
            off += int(ch[b])
        for b in blocks:
            self_col[b] = off
            off += 1
        info.append(dict(blocks=blocks, Lg=Lg, Hg=Hg, Cg=off, cbase=cbase,
                         lo_cols=lo_cols, hi_cols=hi_cols, self_col=self_col,
                         lo_icol=lo_ch_cum * 8, hi_icol=hi_ch_cum * 8))
        cbase += off
        lo_ch_cum += Lg
        hi_ch_cum += Hg
    # pad to a multiple of 8 f32 columns: per-partition row stride must stay
    # 16/32-byte aligned or the staged-input DMA faults on device
    nchunks = (cbase + 7) // 8 * 8
    n_lo_ch, n_hi_ch = lo_ch_cum, hi_ch_cum

    # per-block slot base inside the per-half idx streams
    lo_slot_base = np.concatenate([[0], np.cumsum(cl)]) * P
    hi_slot_base = np.concatenate([[0], np.cumsum(ch)]) * P
    # msg column of (b, half) chunks, global chunk order for mdst/mnorm
    colof = {}
    for g in info:
        for b in g['blocks']:
            colof[(b, 0)] = g['cbase'] + g['lo_cols'][b][0]
            colof[(b, 1)] = g['cbase'] + g['hi_cols'][b][0]
            colof[(b, 2)] = g['cbase'] + g['self_col'][b]

    order_half = np.argsort(hi_s * N * 2 + core_d * npc + blk_d, kind='stable')

    cores = []
    for c in range(NCORES):
        mask = core_d == c
        e_row = row_s[mask]
        e_hi = hi_s[mask]
        e_blk = blk_d[mask]
        e_dl = dl_d[mask]
        e_nrm = nrm_e[mask]

        idx_lo = np.zeros(n_lo_ch * P, np.int64)
        idx_hi = np.zeros(n_hi_ch * P, np.int64)
        mdst = np.zeros((P, nchunks), np.float32)
        mnorm = np.zeros((P, nchunks), np.float32)

        for h, idx_arr, sbase, c_arr in ((0, idx_lo, lo_slot_base, cl),
                                         (1, idx_hi, hi_slot_base, ch)):
            sel = e_hi == (h == 1)
            rr, bb = e_row[sel], e_blk[sel]
            dd, nn = e_dl[sel], e_nrm[sel]
            o = np.argsort(bb, kind='stable')
            rr, bb, dd, nn = rr[o], bb[o], dd[o], nn[o]
            start = np.searchsorted(bb, np.arange(nb))
            end = np.searchsorted(bb, np.arange(nb) + 1)
            for b in range(nb):
                k = end[b] - start[b]
                assert k <= c_arr[b] * P
                sl = slice(start[b], end[b])
                idx_arr[sbase[b]: sbase[b] + k] = rr[sl]
                col0 = colof[(b, h)]
                fd = np.zeros(c_arr[b] * P, np.float32)
                fn = np.zeros(c_arr[b] * P, np.float32)
                fd[:k] = dd[sl]
                fn[:k] = nn[sl]
                mdst[:, col0:col0 + c_arr[b]] = fd.reshape(c_arr[b], P).T
                mnorm[:, col0:col0 + c_arr[b]] = fn.reshape(c_arr[b], P).T

        # self-loop diagonal chunks
        for b in range(nb):
            col = colof[(b, 2)]
            mdst[:, col] = np.arange(P, dtype=np.float32)
            n_real = min(P, npc_raw - b * P)
            nodes = c * npc_raw + b * P + np.arange(n_real)
            d2 = (dinv[nodes] ** 2).astype(np.float32)
            mnorm[:n_real, col] = d2

        assert idx_lo.min() >= 0 and idx_lo.max() < ntot_lo
        assert idx_hi.min() >= 0 and (n_hi_ch == 0 or idx_hi.max() < ntot_hi)

        # x slice, padded/transposed/group-packed: per phase-A group of GA
        # blocks, [P, kt*wg] contiguous (32 KB/partition descriptors)
        nodes = np.arange(c * npc_raw, (c + 1) * npc_raw)
        xp = np.zeros((npc, F_in), np.float32)
        xp[:npc_raw] = np.asarray(x[nodes], np.float32)
        xt3 = np.ascontiguousarray(xp.T).reshape(kt, P, npc)
        parts = []
        for g0 in range(0, nb, GA):
            c0, c1 = g0 * P, min(g0 + GA, nb) * P
            parts.append(xt3[:, :, c0:c1].transpose(1, 0, 2).reshape(P, -1))
        xt = np.concatenate(parts, axis=1).astype(ml_dtypes.bfloat16)

        cores.append({
            "xt": xt,
            "idx_lo": _swizzle_idx(idx_lo),
            "idx_hi": _swizzle_idx(idx_hi),
            "mdst": mdst,
            "mnorm": mnorm,
        })

    iota = np.tile(np.arange(P, dtype=np.float32)[None, :], (P, 1))
    if os.environ.get("GNN_F32_IOTA", "0") != "1":
        iota = iota.astype(ml_dtypes.bfloat16)
    shared = {
        "w1": np.asarray(W1, np.float32).astype(ml_dtypes.bfloat16),
        "w2": np.asarray(W2, np.float32),
        "b1p": np.asarray(b1, np.float32).reshape(2, P).T.copy(),
        "b2b": np.tile(np.asarray(b2, np.float32)[None, :], (P, 1)),
        "iota": iota,
    }
    cfg = dict(N=N, F_in=F_in, F_h=F_h, F_out=F_out, npc_raw=npc_raw, nb=nb,
               nb_lo=nb_lo, nb_hi=nb_hi, npc=npc, npc_lo=npc_lo,
               npc_hi=npc_hi, ntot_lo=ntot_lo, ntot_hi=ntot_hi, kt=kt,
               nchunks=nchunks, n_lo_ch=n_lo_ch, n_hi_ch=n_hi_ch, info=info,
               cl=cl.tolist(), ch=ch.tolist())
    return cfg, cores, shared


# ---------------------------------------------------------------------------
# device kernel
# ---------------------------------------------------------------------------

def _build_nc(cfg):
    F_in, F_h, F_out = cfg["F_in"], cfg["F_h"], cfg["F_out"]
    nb, nb_lo, npc = cfg["nb"], cfg["nb_lo"], cfg["npc"]
    npc_lo, npc_hi = cfg["npc_lo"], cfg["npc_hi"]
    ntot_lo, ntot_hi = cfg["ntot_lo"], cfg["ntot_hi"]
    kt, nchunks = cfg["kt"], cfg["nchunks"]
    n_lo_ch, n_hi_ch = cfg["n_lo_ch"], cfg["n_hi_ch"]
    info = cfg["info"]
    rg = [list(range(NCORES))]

    nc = bacc.Bacc(None, num_devices=NCORES, num_swdge_queues=4)

    xt_d = nc.declare_dram_parameter("xt", [P, kt * npc], BF16, isOutput=False)
    w1_d = nc.declare_dram_parameter("w1", [F_in, F_h], BF16, isOutput=False)
    w2_d = nc.declare_dram_parameter("w2", [F_h, F_out], F32, isOutput=False)
    b1_d = nc.declare_dram_parameter("b1p", [P, 2], F32, isOutput=False)
    b2_d = nc.declare_dram_parameter("b2b", [P, F_out], F32, isOutput=False)
    no_tables = os.environ.get("GNN_NO_TABLES", "0") == "1"
    tab_sel = set(os.environ.get("GNN_TABLES",
                                 "iota,ilo,ihi,mdst,mnorm").split(","))
    IOTA_DT = F32 if os.environ.get("GNN_F32_IOTA", "0") == "1" else BF16
    if not no_tables:
        if "iota" in tab_sel:
            iota_d = nc.declare_dram_parameter("iota", [P, P], IOTA_DT,
                                               isOutput=False)
        if "ilo" in tab_sel:
            ilo_d = nc.declare_dram_parameter("idx_lo", [P, n_lo_ch * 8], I16,
                                              isOutput=False)
        if "ihi" in tab_sel:
            ihi_d = nc.declare_dram_parameter("idx_hi", [P, max(n_hi_ch, 1) * 8],
                                              I16, isOutput=False)
        if "mdst" in tab_sel:
            mdst_d = nc.declare_dram_parameter("mdst", [P, nchunks], F32,
                                               isOutput=False)
        if "mnorm" in tab_sel:
            mnorm_d = nc.declare_dram_parameter("mnorm", [P, nchunks], F32,
                                                isOutput=False)
    out_d = nc.declare_dram_parameter("out", [npc, F_out], F32, isOutput=True)

    with tile.TileContext(nc) as tc:
        with (
            tc.tile_pool(name="const", bufs=1) as const,
            tc.tile_pool(name="dram", bufs=1, space="DRAM") as dram,
        ):
            h_own_l = dram.tile([npc_lo, F_h], BF16)
            h_own_h = dram.tile([npc_hi, F_h], BF16)
            h_full_l = dram.tile([ntot_lo, F_h], BF16, addr_space="Shared")
            h_full_h = dram.tile([ntot_hi, F_h], BF16, addr_space="Shared")
            g_own_l = dram.tile([npc_lo, F_out], BF16)
            g_own_h = dram.tile([npc_hi, F_out], BF16)
            g_full_l = dram.tile([ntot_lo, F_out], BF16, addr_space="Shared")
            g_full_h = dram.tile([ntot_hi, F_out], BF16, addr_space="Shared")

            w1_t = const.tile([P, kt, F_h], BF16)
            w2_t = const.tile([P, 2, F_out], F32)
            b1_t = const.tile([P, 2], F32)
            b2_t = const.tile([P, F_out], F32)
            if not no_tables:
                if "iota" in tab_sel:
                    iota_t = const.tile([P, P], IOTA_DT)
                if "ilo" in tab_sel:
                    ilo_t = const.tile([P, n_lo_ch * 8], I16)
                if "ihi" in tab_sel:
                    ihi_t = const.tile([P, max(n_hi_ch, 1) * 8], I16)
                if "mdst" in tab_sel:
                    mdst_t = const.tile([P, nchunks], F32)
                if "mnorm" in tab_sel:
                    mnorm_t = const.tile([P, nchunks], F32)

            nc.sync.dma_start(w1_t[:], w1_d[:].rearrange("(a p) o -> p a o", p=P))
            nc.sync.dma_start(w2_t[:], w2_d[:].rearrange("(h p) o -> p h o", p=P))
            nc.sync.dma_start(b1_t[:], b1_d[:])
            nc.sync.dma_start(b2_t[:], b2_d[:])
            if not no_tables:
                if "iota" in tab_sel:
                    nc.sync.dma_start(iota_t[:], iota_d[:])
                if "ilo" in tab_sel:
                    nc.sync.dma_start(ilo_t[:], ilo_d[:])
                if "ihi" in tab_sel:
                    nc.sync.dma_start(ihi_t[:], ihi_d[:])
                if "mdst" in tab_sel:
                    nc.sync.dma_start(mdst_t[:], mdst_d[:])
                if "mnorm" in tab_sel:
                    nc.sync.dma_start(mnorm_t[:], mnorm_d[:])


            def maybe_ag(in_t, out_t):
                if os.environ.get("GNN_NO_CC", "0") == "1":
                    return
                nc.gpsimd.collective_compute(
                    "AllGather", mybir.AluOpType.bypass, replica_groups=rg,
                    ins=[in_t[:]], outs=[out_t[:]])
            def h_dest(b):
                if b < nb_lo:
                    return h_own_l[b * P:(b + 1) * P, :]
                return h_own_h[(b - nb_lo) * P:(b - nb_lo + 1) * P, :]

            def g_dest(b):
                if b < nb_lo:
                    return g_own_l[b * P:(b + 1) * P, :]
                return g_own_h[(b - nb_lo) * P:(b - nb_lo + 1) * P, :]

            # ---- phase A: h = x @ W1 (one 4 MB contiguous DMA per group) ----
            psumA = tc.tile_pool(name="psumA", bufs=1, space="PSUM")
            psum = psumA.__enter__()
            workA = tc.tile_pool(name="workA", bufs=1)
            wa = workA.__enter__()
            ag_a_done = False
            off = 0
            for g0 in range(0, nb, GA):
                gb = list(range(g0, min(g0 + GA, nb)))
                wg = len(gb) * P
                xt_t = wa.tile([P, kt * wg], BF16,
                               tag=f"xt{len(gb)}", bufs=3 if len(gb) == GA else 1)
                nc.sync.dma_start(xt_t[:], xt_d[:, off:off + kt * wg])
                phs = [psum.tile([P, F_h], F32, tag="ph", bufs=2 * GA,
                                 space="PSUM", name=f"ph{g0}_{i}")
                       for i in range(len(gb))]
                for a in range(kt):
                    for i in range(len(gb)):
                        nc.tensor.matmul(phs[i][:],
                                         lhsT=xt_t[:, a * wg + i * P:
                                                   a * wg + (i + 1) * P],
                                         rhs=w1_t[:, a, :],
                                         start=(a == 0), stop=(a == kt - 1))
                for i, b in enumerate(gb):
                    h_sb = wa.tile([P, F_h], BF16, tag="hsb", bufs=3)
                    nc.vector.tensor_copy(h_sb[:], phs[i][:])
                    nc.sync.dma_start(h_dest(b), h_sb[:])
                off += kt * wg
                if (not ag_a_done and gb[-1] >= nb_lo - 1
                        and os.environ.get("GNN_NO_MID_AG", "0") != "1"):
                    maybe_ag(h_own_l, h_full_l)
                    ag_a_done = True
            workA.__exit__(None, None, None)
            psumA.__exit__(None, None, None)
            if not ag_a_done:
                maybe_ag(h_own_l, h_full_l)
            maybe_ag(h_own_h, h_full_h)

            psumC = tc.tile_pool(name="psumC", bufs=1, space="PSUM")
            psum = psumC.__enter__()
            workC = tc.tile_pool(name="workC", bufs=1)
            wc = workC.__enter__()
            qn = [0]

            def gathers(dst_t, c0, nch, table, idx_t, icol0, elem):
                for s in range(0, nch, GMAX):
                    k = min(GMAX, nch - s)
                    nc.gpsimd.dma_gather(
                        out_ap=dst_t[:, c0 + s:c0 + s + k, :], in_ap=table,
                        idxs_ap=idx_t[:, icol0 + s * 8:icol0 + (s + k) * 8],
                        num_idxs=k * P, num_idxs_reg=k * P, elem_size=elem,
                        queue_num=qn[0] % 4)
                    qn[0] += 1

            def m_tile(gc):
                m = wc.tile([P, P], BF16, tag="m", bufs=8)
                nc.vector.tensor_scalar(
                    out=m[:], in0=iota_t[:],
                    scalar1=mdst_t[:, gc:gc + 1], scalar2=mnorm_t[:, gc:gc + 1],
                    op0=mybir.AluOpType.is_equal, op1=mybir.AluOpType.mult)
                return m

            # ---- phase C: aggregate layer 1, relu, transform by W2 ----
            skip_c = os.environ.get("GNN_SKIP_C", "0") == "1"
            skip_e = os.environ.get("GNN_SKIP_E", "0") == "1"
            ag2_a_done = False
            for g in (() if skip_c else info):
                blocks, Cg, cb = g["blocks"], g["Cg"], g["cbase"]
                msg = wc.tile([P, Cg, F_h], BF16, tag="msg", bufs=5,
                              name=f"msg{blocks[0]}")
                gathers(msg, 0, g["Lg"], h_full_l[:], ilo_t, g["lo_icol"], F_h)
                gathers(msg, g["Lg"], g["Hg"], h_full_h[:], ihi_t,
                        g["hi_icol"], F_h)
                for b in blocks:
                    nc.sync.dma_start(msg[:, g["self_col"][b], :], h_dest(b))
                for b in blocks:
                    pa = psum.tile([P, P], F32, tag="pa", bufs=2, space="PSUM")
                    pb = psum.tile([P, P], F32, tag="pb", bufs=2, space="PSUM")
                    l0, lk = g["lo_cols"][b]
                    h0, hk = g["hi_cols"][b]
                    chunks = (list(range(l0, l0 + lk)) +
                              list(range(h0, h0 + hk)) + [g["self_col"][b]])
                    for ci, ccol in enumerate(chunks):
                        m = m_tile(cb + ccol)
                        st, sp = ci == 0, ci == len(chunks) - 1
                        nc.tensor.matmul(pa[:], lhsT=msg[:, ccol, 0:P],
                                         rhs=m[:], start=st, stop=sp)
                        nc.tensor.matmul(pb[:], lhsT=msg[:, ccol, P:F_h],
                                         rhs=m[:], start=st, stop=sp)
                    ra = wc.tile([P, P], F32, tag="ra", bufs=2)
                    rb = wc.tile([P, P], F32, tag="rb", bufs=2)
                    nc.scalar.activation(ra[:], pa[:],
                                         mybir.ActivationFunctionType.Relu,
                                         bias=b1_t[:, 0:1], scale=1.0)
                    nc.scalar.activation(rb[:], pb[:],
                                         mybir.ActivationFunctionType.Relu,
                                         bias=b1_t[:, 1:2], scale=1.0)
                    pg = psum.tile([P, F_out], F32, tag="pgo", bufs=2,
                                   space="PSUM")
                    nc.tensor.matmul(pg[:], lhsT=ra[:], rhs=w2_t[:, 0, :],
                                     start=True, stop=False)
                    nc.tensor.matmul(pg[:], lhsT=rb[:], rhs=w2_t[:, 1, :],
                                     start=False, stop=True)
                    g_sb = wc.tile([P, F_out], BF16, tag="gsb", bufs=3)
                    nc.vector.tensor_copy(g_sb[:], pg[:])
                    nc.sync.dma_start(g_dest(b), g_sb[:])
                if (not ag2_a_done and blocks[-1] >= nb_lo - 1
                        and os.environ.get("GNN_NO_MID_AG", "0") != "1"):
                    maybe_ag(g_own_l, g_full_l)
                    ag2_a_done = True
            if not ag2_a_done:
                maybe_ag(g_own_l, g_full_l)
            maybe_ag(g_own_h, g_full_h)

            # ---- phase E: aggregate layer 2, add bias, write out ----
            for g in (() if skip_e else info):
                blocks, Cg, cb = g["blocks"], g["Cg"], g["cbase"]
                msg2 = wc.tile([P, Cg, F_out], BF16, tag="msg2", bufs=6,
                               name=f"msg2_{blocks[0]}")
                gathers(msg2, 0, g["Lg"], g_full_l[:], ilo_t, g["lo_icol"],
                        F_out)
                gathers(msg2, g["Lg"], g["Hg"], g_full_h[:], ihi_t,
                        g["hi_icol"], F_out)
                for b in blocks:
                    nc.sync.dma_start(msg2[:, g["self_col"][b], :], g_dest(b))
                for b in blocks:
                    po = psum.tile([P, F_out], F32, tag="pgo", bufs=2,
                                   space="PSUM")
                    l0, lk = g["lo_cols"][b]
                    h0, hk = g["hi_cols"][b]
                    chunks = (list(range(l0, l0 + lk)) +
                              list(range(h0, h0 + hk)) + [g["self_col"][b]])
                    for ci, ccol in enumerate(chunks):
                        m = m_tile(cb + ccol)
                        nc.tensor.matmul(po[:], lhsT=m[:], rhs=msg2[:, ccol, :],
                                         start=(ci == 0),
                                         stop=(ci == len(chunks) - 1))
                    o_sb = wc.tile([P, F_out], F32, tag="osb", bufs=3)
                    nc.vector.tensor_tensor(out=o_sb[:], in0=po[:], in1=b2_t[:],
                                            op=mybir.AluOpType.add)
                    nc.sync.dma_start(out_d[b * P:(b + 1) * P, :], o_sb[:])
            if skip_e:
                for b in range(nb):
                    o_sb = wc.tile([P, F_out], F32, tag="osb", bufs=3)
                    nc.vector.tensor_copy(o_sb[:], b2_t[:])
                    nc.sync.dma_start(out_d[b * P:(b + 1) * P, :], o_sb[:])
            workC.__exit__(None, None, None)
            psumC.__exit__(None, None, None)

    nc.compile()
    return nc


def _in_maps(cfg, cores, shared):
    maps = [{**shared, **c} for c in cores]
    sel = set(os.environ.get("GNN_TABLES", "iota,ilo,ihi,mdst,mnorm").split(","))
    keymap = {"iota": "iota", "ilo": "idx_lo", "ihi": "idx_hi",
              "mdst": "mdst", "mnorm": "mnorm"}
    for m in maps:
        for short, key in keymap.items():
            if os.environ.get("GNN_NO_TABLES", "0") == "1" or short not in sel:
                m.pop(key, None)
    return maps


def _assemble(cfg, outs):
    N, F_out, npc_raw = cfg["N"], cfg["F_out"], cfg["npc_raw"]
    full = np.empty((N, F_out), np.float32)
    for c in range(NCORES):
        full[c * npc_raw:(c + 1) * npc_raw] = outs[c][:npc_raw]
    return full


# ---------------------------------------------------------------------------
# entry points
# ---------------------------------------------------------------------------

def kernel(x, edge_index, W1, b1, W2, b2):
    cfg, cores, shared = _prep(x, edge_index, W1, b1, W2, b2)
    nc = _build_nc(cfg)
    from concourse.bass_utils import run_bass_kernel_spmd
    res = run_bass_kernel_spmd(nc, _in_maps(cfg, cores, shared),
                               list(range(NCORES)))
    return _assemble(cfg, [r["out"] for r in res.results])


def run_profiled(x, edge_index, W1, b1, W2, b2, tmpdir=None):
    """Like kernel(), but traces on HW; returns (out, exec_time_ns, tmpdir)."""
    import time

    t0 = time.time()
    cfg, cores, shared = _prep(x, edge_index, W1, b1, W2, b2)
    print(f"prep {time.time() - t0:.1f}s; chunks={cfg['nchunks']} "
          f"lo_ch={cfg['n_lo_ch']} hi_ch={cfg['n_hi_ch']}")
    t0 = time.time()
    nc = _build_nc(cfg)
    print(f"build {time.time() - t0:.1f}s; {len(nc.inst_map)} instructions")
    from concourse.bass_utils import run_bass_kernel_spmd
    in_maps = _in_maps(cfg, cores, shared)
    t0 = time.time()
    res = run_bass_kernel_spmd(nc, in_maps, list(range(NCORES)))
    print(f"run {time.time() - t0:.1f}s")
    out = _assemble(cfg, [r["out"] for r in res.results])
    exec_ns = None
    try:
        t0 = time.time()
        res2 = run_bass_kernel_spmd(nc, in_maps, list(range(NCORES)),
                                    trace=True, tmpdir=tmpdir)
        print(f"traced run {time.time() - t0:.1f}s")
        exec_ns = res2.exec_time_ns
    except Exception as e:
        print(f"trace run failed: {type(e).__name__}: {str(e)[:200]}")
    return out, exec_ns, tmpdir


def _numpy_ref(x, edge_index, W1, b1, W2, b2):
    N = x.shape[0]
    src = np.concatenate([edge_index[0], np.arange(N)])
    dst = np.concatenate([edge_index[1], np.arange(N)])
    deg = np.bincount(dst, minlength=N).astype(np.float64)
    dinv = np.where(deg > 0, 1 / np.sqrt(deg), 0)
    nrm = (dinv[src] * dinv[dst]).astype(np.float32)

    def layer(h, W, b):
        hw = h @ W
        out = np.zeros((N, W.shape[1]), np.float32)
        np.add.at(out, dst, hw[src] * nrm[:, None])
        return out + b

    h = np.maximum(layer(x, W1, b1), 0)
    return layer(h, W2, b2)


def _selftest_sim():
    from concourse import bass_interp
    rng = np.random.default_rng(1)
    N, E, F_in = 2048, 8192, 512
    x = rng.standard_normal((N, F_in), dtype=np.float32)
    ei = rng.integers(0, N, (2, E)).astype(np.int64)
    W1 = (rng.standard_normal((F_in, 256), dtype=np.float32) * F_in ** -0.5)
    W2 = (rng.standard_normal((256, 128), dtype=np.float32) * 256 ** -0.5)
    b1 = rng.standard_normal(256).astype(np.float32) * 0.1
    b2 = rng.standard_normal(128).astype(np.float32) * 0.1

    cfg, cores, shared = _prep(x, ei, W1, b1, W2, b2)
    print("cfg:", {k: v for k, v in cfg.items() if k not in ("info",)})
    nc = _build_nc(cfg)
    print("built; instructions:", len(nc.inst_map))

    sim = bass_interp.MultiCoreSim(nc, NCORES)
    for i, m in enumerate(_in_maps(cfg, cores, shared)):
        for k, v in m.items():
            sim.cores[i].tensor(k)[:] = v
    sim.simulate()
    outs = [np.array(sim.cores[i].mem_tensor("out")) for i in range(NCORES)]
    got = _assemble(cfg, outs)
    want = _numpy_ref(x, ei, W1, b1, W2, b2)
    err = np.abs(got - want).max() / (np.abs(want).max() + 1e-9)
    print("selftest rel err:", err)
    assert err < 1e-2, "selftest FAILED"
    print("SELFTEST PASSED")


if __name__ == "__main__":
    _selftest_sim()
